# revision 32
# baseline (speedup 1.0000x reference)
"""GraphTransformer (TransformerConv + mean-pool) on 8 trn2 NeuronCores.

Strategy (two launches):
  Launch A (per core, 6250 nodes + pad -> 6272):
      qkv/skip = x @ fused(W_emb,[Wq|Wk|Wv|Wskip]) + fused bias   (bf16 TensorE)
  Host: assemble Q,K,V (cast fp8), sort nodes by dst-degree, stripe across
      8 cores into 49 tiles x 128 lanes; chunk c of a tile holds the c-th
      incident edge of each lane's node (pad rows zero).  Per-tile chunk
      count C_t = max degree in the tile's degree window (SPMD-identical
      across cores).  Ship per-edge K rows transposed (kgT, channel on
      partitions) and V rows channel-major (vg), plus Q transposed per tile.
  Launch B (per core, 49 tiles, passes of <=PCAP chunks):
      qk   = kgT * qT(bcast)                  DVE 2x
      s    = headsel^T @ qk                   TensorE (head-sum over channels)
      w    = exp(s*scale)                     ACT, [8, P*128] only
      w2   = transpose(w) via identity matmul TensorE -> [128, C*8]
      wv   = vg * w2(bcast)                   DVE 2x (c-major keeps stride-1)
      num += I128 @ wv                        TensorE PSUM accumulate
      den  = sum_c w2 - npad;  out = mean_h(num/(8*den)) + skip
      pooled += indng^T @ out                 TensorE per-graph partial
  Host: sum partial pooled over cores, divide by graph node counts.
"""

import math

import numpy as np
import ml_dtypes

import concourse.bass as bass
from concourse import bacc
import concourse.mybir as mybir
import concourse.tile as tile
from concourse import bass_utils
from concourse.bass import ts

BF16 = mybir.dt.bfloat16
F32 = mybir.dt.float32
F8 = mybir.dt.float8e4
NP_BF16 = ml_dtypes.bfloat16
NP_F8 = ml_dtypes.float8_e4m3fn

N, E, B = 50000, 400000, 64
IN_DIM, OUT_DIM, HEADS = 768, 64, 8
HC = HEADS * OUT_DIM  # 512
NCORES = 8
NPC = N // NCORES  # 6250 nodes per core (launch A sharding)
TILES = 49
NPAD = TILES * 128  # 6272
NSLOT = NCORES * NPAD  # 50176
KCH = IN_DIM // 128  # 6 contraction chunks (launch A)
PCAP = 8  # max chunks per pass in launch B

TRACE = False
LAST_EXEC_NS = {}
LAST_TRACE_PATH = {}

_cache = {}


WQK_SCALE = 32.0  # fp8 range lift for the fused q/k weights


def _build_launch_a():
    # qkv/skip computed directly from x with host-fused weights:
    #   W_eff = W_emb @ [Wq|Wk|Wv|Wskip],  b_eff = b_emb @ [..] + [bq|bk|bv|bskip]
    # q/k columns (1024) run as fp8 DoubleRow matmuls (weights pre-scaled by
    # WQK_SCALE, folded back in the bias epilogue); v/skip stay bf16.
    nc = bacc.Bacc("TRN2", debug=False, num_devices=NCORES)
    xT = nc.dram_tensor("xT", [KCH * 128, NPAD], BF16, kind="ExternalInput").ap()
    xT8 = nc.dram_tensor("xT8", [KCH * 128, NPAD], F8, kind="ExternalInput").ap()
    wqk8 = nc.dram_tensor("wqk8", [KCH * 128, 1024], F8, kind="ExternalInput").ap()
    wvs = nc.dram_tensor("wvs", [KCH * 128, 576], BF16, kind="ExternalInput").ap()
    qkv_out = nc.dram_tensor("qkv_out", [NPAD, 1536], F8, kind="ExternalOutput").ap()
    skip_out = nc.dram_tensor("skip_out", [NPAD, OUT_DIM], F32, kind="ExternalOutput").ap()

    MG = 7  # m-tiles per x-load group (49 = 7x7): stream x in, overlap with PE
    MGW = MG * 128
    with tile.TileContext(nc) as tc:
        with (
            tc.tile_pool(name="const", bufs=1) as cpool,
            tc.tile_pool(name="xg", bufs=2) as gpool,
            tc.tile_pool(name="work", bufs=3) as wpool,
            tc.tile_pool(name="psum_qkv", bufs=2, space="PSUM") as pq,
        ):
            wqk8_sb = cpool.tile([128, KCH * 1024], F8)
            wvs_sb = cpool.tile([128, KCH * 576], BF16)
            for k in range(KCH):
                nc.sync.dma_start(wqk8_sb[:, k * 1024:(k + 1) * 1024], wqk8[ts(k, 128), :])
                nc.sync.dma_start(wvs_sb[:, k * 576:(k + 1) * 576], wvs[ts(k, 128), :])
            wqk8_v = wqk8_sb.rearrange("p (k n) -> p k n", k=KCH)

            for g in range(TILES // MG):
                xg_sb = gpool.tile([128, KCH * MGW], BF16, tag="xg")
                xg8_sb = gpool.tile([128, KCH * MGW], F8, tag="xg8")
                for k in range(KCH):
                    nc.sync.dma_start(
                        xg_sb[:, k * MGW:(k + 1) * MGW],
                        xT[ts(k, 128), g * MGW:(g + 1) * MGW])
                    nc.sync.dma_start(
                        xg8_sb[:, k * MGW:(k + 1) * MGW],
                        xT8[ts(k, 128), g * MGW:(g + 1) * MGW])
                xg8_v = xg8_sb.rearrange("p (k m) -> p k m", k=KCH)
                for mm in range(MG):
                    m = g * MG + mm
                    qkvs_ps = pq.tile([128, 1600], F32, tag="qkvs")
                    for kk in range(KCH // 2):
                        for n0 in (0, 512):
                            nc.tensor.matmul(
                                qkvs_ps[:, n0:n0 + 512],
                                lhsT=xg8_v[:, 2 * kk:2 * kk + 2, ts(mm, 128)],
                                rhs=wqk8_v[:, 2 * kk:2 * kk + 2, n0:n0 + 512],
                                start=(kk == 0),
                                stop=(kk == KCH // 2 - 1),
                                perf_mode=mybir.MatmulPerfMode.DoubleRow,
                            )
                    for k in range(KCH):
                        for n0, nw in ((0, 512), (512, 64)):
                            nc.tensor.matmul(
                                qkvs_ps[:, 1024 + n0:1024 + n0 + nw],
                                lhsT=xg_sb[:, k * MGW + mm * 128: k * MGW + (mm + 1) * 128],
                                rhs=wvs_sb[:, k * 576 + n0: k * 576 + n0 + nw],
                                start=(k == 0),
                                stop=(k == KCH - 1),
                            )
                    qkv_sb = wpool.tile([128, 1536], F8, tag="qkv")
                    nc.scalar.copy(qkv_sb[:], qkvs_ps[:, :1536])
                    skip_sb = wpool.tile([128, OUT_DIM], F32, tag="skip")
                    nc.scalar.copy(skip_sb[:], qkvs_ps[:, 1536:1600])
                    nc.sync.dma_start(qkv_out[ts(m, 128), :], qkv_sb[:])
                    nc.sync.dma_start(skip_out[ts(m, 128), :], skip_sb[:])
    nc.compile()
    return nc


def _passes(c):
    return [(p0, min(PCAP, c - p0)) for p0 in range(0, c, PCAP)]


def _build_launch_b(c_list):
    nc = bacc.Bacc("TRN2", debug=False, num_devices=NCORES)
    cmax = max(max(c_list), 1)
    assert min(c_list) > 0, "epilogue batching assumes no empty tiles"
    free_tot = sum(512 + c * 1024 for c in c_list)
    ngrp = (TILES + 3) // 4
    kvq = nc.dram_tensor("kvq", [128, free_tot], F8, kind="ExternalInput").ap()
    sknp = nc.dram_tensor("sknp", [ngrp, 128, 4 * 66], F32, kind="ExternalInput").ap()
    indng = nc.dram_tensor("indng", [ngrp, 128, 4 * B], BF16, kind="ExternalInput").ap()
    hsel = nc.dram_tensor("hsel", [128, 8], BF16, kind="ExternalInput").ap()
    identt = nc.dram_tensor("identt", [128, 128], BF16, kind="ExternalInput").ap()
    i8t = nc.dram_tensor("i8t", [8, 8], BF16, kind="ExternalInput").ap()
    pooled = nc.dram_tensor("pooled", [B, OUT_DIM], F32, kind="ExternalOutput").ap()

    scale = 1.0 / math.sqrt(float(OUT_DIM))

    with tile.TileContext(nc) as tc:
        with (
            tc.tile_pool(name="const", bufs=1) as cp,
            tc.tile_pool(name="io", bufs=6) as iop,
            tc.tile_pool(name="tio", bufs=4) as tiop,
            tc.tile_pool(name="wk", bufs=3) as wk,
            tc.tile_pool(name="ps_s", bufs=3, space="PSUM") as ps_s,
            tc.tile_pool(name="ps_w2", bufs=2, space="PSUM") as ps_w2,
            tc.tile_pool(name="ps_num", bufs=2, space="PSUM") as ps_num,
            tc.tile_pool(name="ps_pool", bufs=1, space="PSUM") as ps_pool,
            tc.tile_pool(name="outp", bufs=1) as outp,
        ):
            hsel_sb = cp.tile([128, 8], BF16)
            nc.sync.dma_start(hsel_sb[:], hsel[:])
            ident_sb = cp.tile([128, 128], BF16)
            nc.sync.dma_start(ident_sb[:], identt[:])
            i8_sb = cp.tile([8, 8], BF16)
            nc.sync.dma_start(i8_sb[:], i8t[:])

            pool_ps = ps_pool.tile([B, OUT_DIM], F32)
            GEP = 4  # tiles per batched epilogue group
            scale_f = float(scale)

            # flatten (tile, pass) with kvq offsets; software-pipeline stages
            # across passes so each engine's queue runs without intra-pass waits
            plist = []
            qt_off = {}
            off = 0
            for t, C in enumerate(c_list):
                qt_off[t] = off
                off += 512
                for (c0, P) in _passes(C):
                    plist.append((t, C, c0, P, off))
                    off += P * 1024
            NPI = len(plist)
            kvs, qks, wTs, wvs = {}, {}, {}, {}
            qts, w2sbs, numps = {}, {}, {}
            grp = {}

            def stage_dma(i):
                t, C, c0, P, o = plist[i]
                if c0 == 0:
                    if t % GEP == 0:
                        g = t // GEP
                        sknpw = tiop.tile([128, GEP * 66], F32, tag="sknpw")
                        nc.sync.dma_start(sknpw[:], sknp[g])
                        indngw = tiop.tile([128, GEP * B], BF16, tag="ing")
                        nc.sync.dma_start(indngw[:], indng[g])
                        denw = wk.tile([128, GEP * 8], F32, tag="denw")
                        numw = wk.tile([128, GEP * HC], BF16, tag="numw")
                        grp[g] = (sknpw, indngw, denw, numw)
                    qt = tiop.tile([128, 512], BF16, tag="qt")
                    nc.gpsimd.dma_start(qt[:], kvq[:, qt_off[t]:qt_off[t] + 512])
                    qts[t] = qt
                    w2sbs[t] = wk.tile([128, cmax * 8], BF16, tag="w2", name="w2sb")
                kv = iop.tile([128, PCAP * 1024], BF16, tag="kv")
                nc.gpsimd.dma_start(kv[:, :P * 1024], kvq[:, o:o + P * 1024])
                kvs[i] = kv

            def stage_qk(i):
                t, C, c0, P, o = plist[i]
                kg4 = kvs[i][:, :P * 512].rearrange("p (b c d) -> p b c d", b=4, c=P)
                qk_sb = wk.tile([128, PCAP * 512], BF16, tag="qk")
                nc.vector.tensor_mul(
                    qk_sb[:, :P * 512].rearrange("p (b c d) -> p b c d", b=4, c=P),
                    kg4,
                    qts[t].rearrange("p (b d) -> p b () d", b=4).to_broadcast(
                        [128, 4, P, 128]),
                )
                qks[i] = qk_sb

            def stage_s(i):
                t, C, c0, P, o = plist[i]
                qk4 = qks[i][:, :P * 512].rearrange("p (b c d) -> p b c d", b=4, c=P)
                wT_sb = wk.tile([8, PCAP * 128], BF16, tag="wT")
                for p0 in range(0, P, 4):
                    pw = min(4, P - p0)
                    s_ps = ps_s.tile([8, 512], F32, tag="s")
                    for bb in range(4):
                        nc.tensor.matmul(
                            s_ps[:, :pw * 128],
                            lhsT=hsel_sb[:],
                            rhs=qk4[:, bb, p0:p0 + pw],
                            start=(bb == 0),
                            stop=(bb == 3),
                        )
                    nc.scalar.activation(
                        out=wT_sb[:, p0 * 128:(p0 + pw) * 128],
                        in_=s_ps[:, :pw * 128],
                        func=mybir.ActivationFunctionType.Exp,
                        scale=scale_f,
                    )
                wTs[i] = wT_sb

            def stage_t(i):
                t, C, c0, P, o = plist[i]
                w2_ps = ps_w2.tile([128, PCAP * 8], F32, tag="w2ps")
                for c in range(P):
                    nc.tensor.matmul(
                        w2_ps[:, c * 8:(c + 1) * 8],
                        lhsT=wTs[i][:, c * 128:(c + 1) * 128],
                        rhs=i8_sb[:],
                        start=True,
                        stop=True,
                    )
                nc.scalar.copy(
                    w2sbs[t][:, c0 * 8:(c0 + P) * 8], w2_ps[:, :P * 8])

            def stage_wv(i):
                t, C, c0, P, o = plist[i]
                wv_sb = wk.tile([128, PCAP * 512], BF16, tag="wv")
                nc.vector.tensor_mul(
                    wv_sb[:, :P * 512].rearrange("p (c f h) -> p c f h", c=P, f=OUT_DIM),
                    kvs[i][:, P * 512:P * 1024].rearrange(
                        "p (c f h) -> p c f h", c=P, f=OUT_DIM),
                    w2sbs[t][:, c0 * 8:(c0 + P) * 8].rearrange(
                        "p (c h) -> p c () h", c=P).to_broadcast(
                        [128, P, OUT_DIM, HEADS]),
                )
                wvs[i] = wv_sb

            def stage_num(i):
                wv_sb = wvs.pop(i)
                t, C, c0, P, o = plist[i]
                if c0 == 0:
                    numps[t] = ps_num.tile([128, HC], F32, tag="num", name="num_ps")
                for c in range(P):
                    nc.tensor.matmul(
                        numps[t][:],
                        lhsT=ident_sb[:],
                        rhs=wv_sb[:, c * 512:(c + 1) * 512],
                        start=(c0 + c == 0),
                        stop=(c0 + c == C - 1),
                    )
                if c0 + P < C:
                    return
                # tile complete: den reduce + num evac into the group buffers
                g, gi = t // GEP, t % GEP
                sknpw, indngw, denw, numw = grp[g]
                nc.vector.reduce_sum(
                    denw[:, gi * 8:(gi + 1) * 8],
                    w2sbs[t][:, :C * 8].rearrange("p (c h) -> p h c", c=C),
                    axis=mybir.AxisListType.X,
                )
                nc.scalar.copy(numw[:, gi * HC:(gi + 1) * HC], numps[t][:])
                if gi != GEP - 1 and t != TILES - 1:
                    return
                # group epilogue
                nt = gi + 1
                t0 = t - gi
                sknpw3 = sknpw.rearrange("p (t f) -> p t f", t=GEP)
                recw = wk.tile([128, GEP * 8], F32, tag="recw")
                nc.vector.tensor_sub(
                    recw[:, :nt * 8].rearrange("p (t h) -> p t h", t=nt),
                    denw[:, :nt * 8].rearrange("p (t h) -> p t h", t=nt),
                    sknpw3[:, :nt, 64:65].to_broadcast([128, nt, 8]),
                )
                nc.vector.tensor_scalar(
                    out=recw[:, :nt * 8], in0=recw[:, :nt * 8],
                    scalar1=float(HEADS), scalar2=1e-12,
                    op0=mybir.AluOpType.mult, op1=mybir.AluOpType.add,
                )
                recb = wk.tile([128, GEP * 8], BF16, tag="recb")
                with nc.allow_low_precision(reason="bf16 1/den keeps the mh multiply in DVE 2x mode"):
                    nc.vector.reciprocal(recb[:, :nt * 8], recw[:, :nt * 8])
                mhw = wk.tile([128, GEP * HC], BF16, tag="mhw")
                nc.vector.tensor_mul(
                    mhw[:, :nt * HC].rearrange("p (t f h) -> p t f h", t=nt, f=OUT_DIM),
                    numw[:, :nt * HC].rearrange("p (t f h) -> p t f h", t=nt, f=OUT_DIM),
                    recb[:, :nt * 8].rearrange("p (t h) -> p t () h", t=nt).to_broadcast(
                        [128, nt, OUT_DIM, HEADS]),
                )
                msumw = wk.tile([128, GEP * OUT_DIM], F32, tag="msumw")
                nc.vector.reduce_sum(
                    msumw[:, :nt * OUT_DIM].rearrange("p (t f) -> p t f", t=nt),
                    mhw[:, :nt * HC].rearrange("p (t f h) -> p t f h", t=nt, f=OUT_DIM),
                    axis=mybir.AxisListType.X,
                )
                outw = wk.tile([128, GEP * OUT_DIM], BF16, tag="outw")
                nc.vector.tensor_add(
                    outw[:, :nt * OUT_DIM].rearrange("p (t f) -> p t f", t=nt),
                    msumw[:, :nt * OUT_DIM].rearrange("p (t f) -> p t f", t=nt),
                    sknpw3[:, :nt, 0:64],
                )
                for g2 in range(nt):
                    nc.tensor.matmul(
                        pool_ps[:],
                        lhsT=indngw[:, g2 * B:(g2 + 1) * B],
                        rhs=outw[:, g2 * OUT_DIM:(g2 + 1) * OUT_DIM],
                        start=(t0 + g2 == 0), stop=(t0 + g2 == TILES - 1),
                    )

            for i in range(NPI + 5):
                if i < NPI:
                    stage_dma(i)
                if 0 <= i - 4 < NPI:
                    stage_wv(i - 4)
                if 0 <= i - 1 < NPI:
                    stage_qk(i - 1)
                if 0 <= i - 2 < NPI:
                    stage_s(i - 2)
                if 0 <= i - 3 < NPI:
                    stage_t(i - 3)
                if 0 <= i - 5 < NPI:
                    stage_num(i - 5)
                    kvs.pop(i - 5, None)
                    qks.pop(i - 5, None)
                    wTs.pop(i - 5, None)
            pooled_sb = outp.tile([B, OUT_DIM], F32)
            nc.vector.tensor_copy(pooled_sb[:], pool_ps[:])
            nc.sync.dma_start(pooled[:], pooled_sb[:])
    nc.compile()
    return nc


def _get_program_a():
    if "A" not in _cache:
        _cache["A"] = _build_launch_a()
    return _cache["A"]


def _get_program_b(c_list):
    key = ("B", tuple(c_list))
    if key not in _cache:
        _cache[key] = _build_launch_b(c_list)
    return _cache[key]


def _ensure_hook_shim():
    import sys
    import types

    if "antenv.axon_hooks" in sys.modules:
        return
    mod = types.ModuleType("antenv.axon_hooks")
    holder = [None]
    mod.set_axon_ntff_profile_hook = lambda h: holder.__setitem__(0, h)
    mod.get_axon_ntff_profile_hook = lambda: holder[0]
    sys.modules["antenv.axon_hooks"] = mod
    import antenv

    antenv.axon_hooks = mod
    from trn_agent_boot.trn_boot import _ntff_profile_via_ctypes

    mod.set_axon_ntff_profile_hook(
        _ntff_profile_via_ctypes("/opt/axon/libaxon_pjrt.so")
    )


def _run(nc, in_maps, label):
    if not TRACE:
        res = bass_utils.run_bass_kernel_spmd(nc, in_maps, list(range(NCORES)))
        return res.results

    import glob
    import os
    import tempfile

    from concourse import bass2jax
    from concourse._compat import FishPath
    import gauge.profiler

    _ensure_hook_shim()
    import antenv.axon_hooks as hooks

    tmpdir = tempfile.mkdtemp(prefix=f"bass_{label}_")
    with hooks.get_axon_ntff_profile_hook()(tmpdir, [0]):
        results = bass2jax.run_bass_via_pjrt(nc, in_maps, n_cores=NCORES)
    exec_ns = None
    try:
        ntffs = glob.glob(os.path.join(tmpdir, "*_body*.ntff"))
        if ntffs:
            profile = gauge.profiler.Profile(
                profile_path=FishPath(tmpdir),
                kernel_dev_mode=True,
                profile_on_exit=False,
                bass_kernel=nc.m,
                offline_processing=True,
                fname="*_body*",
            )
            prs = profile.to_perfetto(model_index=(0,))
            if prs:
                exec_ns = max(p.exec_time_ns for p in prs)
                LAST_TRACE_PATH[label] = (tmpdir, [p.trace_path for p in prs])
        else:
            print(f"[{label}] no ntff files in {tmpdir}: {os.listdir(tmpdir)}")
    except Exception as e:  # profiling must never break the run
        print(f"[{label}] profile processing failed: {type(e).__name__}: {e}")
    LAST_EXEC_NS[label] = exec_ns
    return results


def kernel(x, edge_index, batch, W_emb, b_emb, Wq, bq, Wk, bk, Wv, bv, Wskip, bskip):
    x = np.asarray(x, np.float32)
    edge_index = np.asarray(edge_index)
    batch_np = np.asarray(batch, np.int64)
    ncA = _get_program_a()

    # ---- host prep for launch A: fold W_emb/b_emb into the qkv/skip weights ----
    wcat = np.concatenate(
        [np.asarray(Wq, np.float32), np.asarray(Wk, np.float32),
         np.asarray(Wv, np.float32), np.asarray(Wskip, np.float32)], axis=1
    )  # [768, 1600]
    bcat = np.concatenate(
        [np.asarray(bq, np.float32), np.asarray(bk, np.float32),
         np.asarray(bv, np.float32), np.asarray(bskip, np.float32)]
    )  # [1600]
    wemb_f = np.asarray(W_emb, np.float32)
    bemb_f = np.asarray(b_emb, np.float32)
    weff = wemb_f @ wcat                              # [768, 1600]
    wqk8 = (weff[:, :1024] * WQK_SCALE).astype(NP_F8)
    wvs = weff[:, 1024:].astype(NP_BF16)              # [768, 576]
    beff = bemb_f @ wcat + bcat                       # [1600] f32

    xpad = np.zeros((NCORES * NPAD, IN_DIM), NP_BF16)
    for c in range(NCORES):
        xpad[c * NPAD: c * NPAD + NPC] = x[c * NPC:(c + 1) * NPC].astype(NP_BF16)
    in_maps_a = []
    for c in range(NCORES):
        xT = np.ascontiguousarray(xpad[c * NPAD:(c + 1) * NPAD].T)  # [768, 6272]
        in_maps_a.append({"xT": xT, "xT8": xT.astype(NP_F8),
                          "wqk8": wqk8, "wvs": wvs})
    res_a = _run(ncA, in_maps_a, "A")

    # ---- host mid: add biases, assemble Q,K,V (fp8) + skip; degree-grouped tiles ----
    qkv8 = np.concatenate([res_a[c]["qkv_out"][:NPC] for c in range(NCORES)])  # [N,1536] fp8 raw
    SKraw = np.concatenate([res_a[c]["skip_out"][:NPC] for c in range(NCORES)])  # [N,64] f32 raw

    bv_mean = beff[1024:1536].reshape(HEADS, OUT_DIM).mean(axis=0)  # v-bias folded: sum_e alpha = 1
    SK = SKraw + (beff[1536:] + bv_mean).astype(np.float32)

    Q8 = np.zeros((N + 1, HC), NP_F8)
    K8 = np.zeros((N + 1, HC), NP_F8)
    V8 = np.zeros((N + 1, HC), NP_F8)
    inv_s = np.float32(1.0 / WQK_SCALE)
    Q8[:N] = (qkv8[:, 0:512].astype(np.float32) * inv_s + beff[:512]).astype(NP_F8)
    K8[:N] = (qkv8[:, 512:1024].astype(np.float32) * inv_s + beff[512:1024]).astype(NP_F8)
    V8[:N] = qkv8[:, 1024:1536]

    src = np.asarray(edge_index[0], np.int64)
    dst = np.asarray(edge_index[1], np.int64)
    deg = np.bincount(dst, minlength=N)
    # deg-0 nodes get no message, so the folded bv_mean must not apply to them
    SK[deg == 0] -= bv_mean

    order = np.argsort(deg, kind="stable")  # ascending degree
    slot_node = np.full(NSLOT, N, np.int64)
    slot_node[176:] = order
    pos_of_node = np.empty(N, np.int64)
    pos_of_node[order] = 176 + np.arange(N)

    degslot = np.zeros(NSLOT, np.int64)
    degslot[176:] = deg[order]
    c_arr = degslot.reshape(TILES, NCORES * 128).max(axis=1)
    # interleave small/big tiles in processing order to smooth the pipeline
    srt = np.argsort(c_arr, kind="stable")
    torder = np.empty(TILES, np.int64)
    torder[0::2] = srt[:(TILES + 1) // 2]
    torder[1::2] = srt[(TILES + 1) // 2:][::-1]
    c_list = [int(c) for c in c_arr[torder]]
    choff = np.zeros(TILES + 1, np.int64)
    choff[1:] = np.cumsum(c_list)
    nch = int(choff[-1])

    # edge -> (core, tile, lane, chunk); tile ids remapped to processing order
    inv_torder = np.empty(TILES, np.int64)
    inv_torder[torder] = np.arange(TILES)
    pd = pos_of_node[dst]
    ecore = pd % NCORES
    er = pd // NCORES
    etile = inv_torder[er // 128]
    elane = er % 128
    o = np.argsort(pd, kind="stable")
    pds = pd[o]
    uniq, grp_start = np.unique(pds, return_index=True)
    starts_per_edge = np.zeros(E, np.int64)
    starts_per_edge[grp_start] = grp_start
    starts_per_edge = np.maximum.accumulate(starts_per_edge)
    k_in_grp = np.arange(E) - starts_per_edge
    kchunk = np.empty(E, np.int64)
    kchunk[o] = k_in_grp

    src_grid = np.full((NCORES, nch, 128), N, np.int32)
    src_grid[ecore, choff[etile] + kchunk, elane] = src

    # node grid per (core, tile, lane)
    posg = (torder[None, :, None] * (NCORES * 128)
            + np.arange(128)[None, None, :] * NCORES
            + np.arange(NCORES)[:, None, None])  # [core, processed-tile, lane]
    node_grid = slot_node[posg]  # [NCORES, TILES, 128]

    SKz = np.zeros((N + 1, OUT_DIM), np.float32)
    SKz[:N] = SK
    degz = np.zeros(N + 1, np.int64)
    degz[:N] = deg
    batchz = np.full(N + 1, -1, np.int64)
    batchz[:N] = batch_np

    ngrp = (TILES + 3) // 4
    sknp_all = np.zeros((NCORES, ngrp * 4, 128, 66), np.float32)
    sknp_all[:, :TILES, :, :64] = SKz[node_grid]
    sknp_all[:, :TILES, :, 64] = (np.asarray(c_list)[None, :, None] - degz[node_grid])
    # group layout: [core, grp, 128, 4*66]
    sknp_all = np.ascontiguousarray(
        sknp_all.reshape(NCORES, ngrp, 4, 128, 66).transpose(0, 1, 3, 2, 4)
    ).reshape(NCORES, ngrp, 128, 4 * 66)
    indng_all = np.zeros((NCORES, ngrp * 4, 128, B), NP_BF16)
    indng_all[:, :TILES] = (
        batchz[node_grid][:, :, :, None] == np.arange(B)[None, None, None, :]
    ).astype(NP_BF16)
    indng_all = np.ascontiguousarray(
        indng_all.reshape(NCORES, ngrp, 4, 128, B).transpose(0, 1, 3, 2, 4)
    ).reshape(NCORES, ngrp, 128, 4 * B)

    hsel_np = (np.arange(128)[:, None] % 8 == np.arange(8)[None, :]).astype(NP_BF16)
    ident_np = np.eye(128, dtype=NP_BF16)
    i8_np = np.eye(8, dtype=NP_BF16)

    free_tot = sum(512 + c * 1024 for c in c_list)
    in_maps_b = []
    for c in range(NCORES):
        KG = K8[src_grid[c]]  # [nch, 128, 512] fp8
        VG = V8[src_grid[c]]
        kvq = np.empty((128, free_tot), NP_F8)
        fo = 0
        for t, C in enumerate(c_list):
            nodes_t = node_grid[c, t]  # [128]
            qrows = Q8[nodes_t]  # [128, 512]
            # qT: [d, h, b, l] -> [l, h, b, d] -> [128, 4*128]
            qT = qrows.reshape(128, 8, 4, 16).transpose(3, 1, 2, 0).reshape(128, 512)
            kvq[:, fo:fo + 512] = qT
            fo += 512
            for (p0, P) in _passes(C):
                kgp = KG[choff[t] + p0: choff[t] + p0 + P]  # [P, 128, 512]
                # kgT: [c, d, h, b, l] -> [l, h, b, c, d]
                kgT = kgp.reshape(P, 128, 8, 4, 16).transpose(4, 2, 3, 0, 1).reshape(
                    128, P * 512)
                kvq[:, fo:fo + P * 512] = kgT
                fo += P * 512
                vgp = VG[choff[t] + p0: choff[t] + p0 + P]  # [P, 128, 512]
                # vg c-major: [c, d, h, f] -> [d, c, f, h]
                vgc = vgp.reshape(P, 128, 8, 64).transpose(1, 0, 3, 2).reshape(
                    128, P * 512)
                kvq[:, fo:fo + P * 512] = vgc
                fo += P * 512
        assert fo == free_tot
        in_maps_b.append({
            "kvq": kvq,
            "sknp": sknp_all[c],
            "indng": indng_all[c],
            "hsel": hsel_np,
            "identt": ident_np,
            "i8t": i8_np,
        })

    ncB = _get_program_b(c_list)
    res_b = _run(ncB, in_maps_b, "B")

    pooled = np.zeros((B, OUT_DIM), np.float64)
    for c in range(NCORES):
        pooled += res_b[c]["pooled"].astype(np.float64)
    cnt = np.bincount(batch_np, minlength=B).astype(np.float64)
    pooled /= np.maximum(cnt, 1.0)[:, None]
    return pooled.astype(np.float32)
